# revision 27
# baseline (speedup 1.0000x reference)
"""Trainium2 Bass kernel for DepthwiseXCorrAug.

Computes, for B=64 samples sharded 8-per-core across 8 NeuronCores:
  k = relu(bn(conv3x3_valid(kernel_in, w_k)))     # [B,256,5,5]
  s = relu(bn(conv3x3_same(search_in, w_s)))      # [B,256,31,31]
  out = per-sample per-channel xcorr(s, k), pad 2 # [B,256,31,31]

Device strategy (per core):
  - conv branches as bf16 matmuls over (ci-block x 3x3-tap) accumulated in
    fp32 PSUM; BN folded into weights on host, bias+ReLU by ScalarE on
    eviction into zero-bordered bf16 spout tiles.
  - depthwise xcorr: the (g0, ob0) quarter runs on the Vector engine as
    per-tap multiply-accumulate (scalar_tensor_tensor) into SBUF; the rest
    runs on the PE as bf16 diagonal-weight matmuls in 64x64 tiling at
    sample-pair granularity.
  - diagonal strips built on GpSimd as ONE broadcast tensor_tensor per
    (sample, ob): kf broadcast (0-stride) x tiled diag mask.
  - DMA: partition-major packed layouts (multi-KB descriptors); outputs
    drain into a packed [g,ob,r,128,961] layout across both HWDGE rings.
"""

import sys

sys.path.insert(0, "/opt/trn_rl_repo")

import numpy as np

import concourse.bass as bass
import concourse.mybir as mybir
import concourse.tile as tile
from concourse import bacc
from concourse.bass_utils import run_bass_kernel_spmd

EPS = 1e-5
N_CORES = 8
B, CIN, HID = 64, 256, 256
SPC = B // N_CORES  # samples per core

_cached_nc = None
last_results = None  # set by kernel(); used by test harness for profiling


def _build_program():
    f32 = mybir.dt.float32
    bf16 = mybir.dt.bfloat16
    RELU = mybir.ActivationFunctionType.Relu
    MULT = mybir.AluOpType.mult
    ADD = mybir.AluOpType.add

    nc = bacc.Bacc("TRN2", target_bir_lowering=False, debug=False,
                   num_devices=N_CORES)

    wTs_d = [nc.dram_tensor(f"wTs{cb}", [128, 2304], bf16,
                            kind="ExternalInput").ap() for cb in range(2)]
    wTk_d = nc.dram_tensor("wTk", [128, 4608], bf16, kind="ExternalInput").ap()
    xk_d = nc.dram_tensor("xk", [128, 4608], bf16, kind="ExternalInput").ap()
    xs_d = nc.dram_tensor("xs", [SPC, 128, 2244], bf16, kind="ExternalInput").ap()
    cst_d = nc.dram_tensor("cst", [128, 4], f32, kind="ExternalInput").ap()
    msk_d = nc.dram_tensor("msk", [128, 1600], bf16, kind="ExternalInput").ap()
    outp_d = nc.dram_tensor("outp", [2, 2, 4, 128, 961], f32,
                            kind="ExternalOutput").ap()

    with tile.TileContext(nc) as tc:
        with tc.tile_pool(name="wp", bufs=1) as wp, \
             tc.tile_pool(name="spin", bufs=8) as spin_pool, \
             tc.tile_pool(name="xop", bufs=10) as xout_pool, \
             tc.tile_pool(name="xov", bufs=4) as xov_pool, \
             tc.tile_pool(name="psc", bufs=4, space="PSUM") as psc, \
             tc.tile_pool(name="psx", bufs=4, space="PSUM") as psx:

            # ---- persistent tiles ----
            wTs = [[wp.tile([128, 1152], bf16, tag=f"wTs{cb}{ob}",
                            name=f"wTs{cb}{ob}") for ob in range(2)]
                   for cb in range(2)]
            wTk = wp.tile([128, 4608], bf16, tag="wTk", name="wTk")
            xk = wp.tile([128, 4608], bf16, tag="xk", name="xk")
            cst = wp.tile([128, 4], f32, tag="cst", name="cst")
            msk = wp.tile([128, 1600], bf16, tag="msk", name="msk")
            kf = [wp.tile([128, 200], f32, tag=f"kf{ob}", name=f"kf{ob}")
                  for ob in range(2)]
            spout = {}
            strips = {}
            for s in range(SPC):
                for ob in range(2):
                    spout[(s, ob)] = wp.tile(
                        [128, 35 * 35], bf16, tag=f"sp{s}_{ob}", name=f"sp{s}_{ob}")
                    if not (ob == 0 and s < 4):  # (g0, ob0) runs on Vector
                        strips[(s, ob)] = wp.tile(
                            [128, 1600], bf16, tag=f"st{s}_{ob}",
                            name=f"st{s}_{ob}")

            # zero spout borders while the first DMAs are in flight
            for s in range(SPC):
                for ob in range(2):
                    nc.gpsimd.memset(spout[(s, ob)][:], 0.0)

            # ---- input DMA: head-critical first on the sync ring ----
            spin = {}

            def alloc_spin(s):
                spin[s] = spin_pool.tile([128, 2244], bf16, tag="spin",
                                         name=f"spin{s}")

            spin0 = [wp.tile([128, 1122], bf16, tag=f"spin0{cb}",
                             name=f"spin0{cb}") for cb in range(2)]
            nc.sync.dma_start(wTs[0][0][:], wTs_d[0][:, 0:1152])
            nc.sync.dma_start(spin0[0][:], xs_d[0][:, 0:1122])
            nc.sync.dma_start(wTs[1][0][:], wTs_d[1][:, 0:1152])
            nc.sync.dma_start(spin0[1][:], xs_d[0][:, 1122:2244])
            nc.sync.dma_start(wTs[0][1][:], wTs_d[0][:, 1152:2304])
            nc.sync.dma_start(cst[:], cst_d)
            nc.sync.dma_start(wTs[1][1][:], wTs_d[1][:, 1152:2304])
            alloc_spin(1)
            nc.sync.dma_start(spin[1][:], xs_d[1])
            nc.scalar.dma_start(wTk[:], wTk_d)
            nc.scalar.dma_start(xk[:], xk_d)
            nc.scalar.dma_start(msk[:], msk_d)
            for s in range(2, SPC):
                alloc_spin(s)
                nc.sync.dma_start(spin[s][:], xs_d[s])

            def spin_view(s, cb):
                if s == 0:
                    return spin0[cb][:].rearrange(
                        "p (h w) -> p h w", h=33, w=34)
                return spin[s][:, cb * 1122:(cb + 1) * 1122].rearrange(
                    "p (h w) -> p h w", h=33, w=34)

            # ---- conv_s for one sample ----
            def conv_s_sample(s):
                for ob in range(2):
                    p0 = psc.tile([128, 512], f32, tag="conv", name=f"c{s}{ob}0")
                    p1 = psc.tile([128, 512], f32, tag="conv", name=f"c{s}{ob}1")
                    idx = 0
                    for cb in range(2):
                        v = spin_view(s, cb)
                        for t in range(9):
                            dy, dx = t // 3, t % 3
                            lhsT = wTs[cb][ob][:, t * 128:(t + 1) * 128]
                            nc.tensor.matmul(
                                p0[:, 0:496], lhsT,
                                v[:, dy:dy + 16, dx:dx + 31],
                                start=(idx == 0), stop=(idx == 17))
                            nc.tensor.matmul(
                                p1[:, 0:465], lhsT,
                                v[:, 16 + dy:16 + dy + 15, dx:dx + 31],
                                start=(idx == 0), stop=(idx == 17))
                            idx += 1
                    sov = spout[(s, ob)][:].rearrange(
                        "p (h w) -> p h w", h=35, w=35)
                    nc.scalar.activation(
                        sov[:, 2:18, 2:33],
                        p0[:, 0:496].rearrange("p (h w) -> p h w", h=16, w=31),
                        RELU, bias=cst[:, 2 + ob:3 + ob], scale=1.0)
                    nc.scalar.activation(
                        sov[:, 18:33, 2:33],
                        p1[:, 0:465].rearrange("p (h w) -> p h w", h=15, w=31),
                        RELU, bias=cst[:, 2 + ob:3 + ob], scale=1.0)

            # ---- conv_k: all 8 samples batched on the free dim ----
            def emit_conv_k():
                for ob in range(2):
                    pk = psc.tile([128, 512], f32, tag="conv", name=f"pk{ob}")
                    idx = 0
                    for cb in range(2):
                        for t in range(9):
                            nc.tensor.matmul(
                                pk[:, 0:200],
                                wTk[:, cb * 2304 + ob * 1152 + t * 128:
                                    cb * 2304 + ob * 1152 + (t + 1) * 128],
                                xk[:, cb * 2304 + t * 256:
                                   cb * 2304 + t * 256 + 200],
                                start=(idx == 0), stop=(idx == 17))
                            idx += 1
                    nc.scalar.activation(kf[ob][:], pk[:, 0:200], RELU,
                                         bias=cst[:, ob:ob + 1], scale=1.0)

            # ---- strips on GpSimd: one broadcast mult per (s, ob) ----
            mskv = msk[:].rearrange("p (t c) -> p t c", t=25, c=64)

            def emit_strips(units):
                for (s, ob) in units:
                    st = strips[(s, ob)]
                    kv = kf[ob][:, s * 25:(s + 1) * 25].unsqueeze(2) \
                        .broadcast_to([128, 25, 64])
                    nc.gpsimd.tensor_tensor(
                        st[:].rearrange("p (t c) -> p t c", t=25, c=64),
                        kv, mskv, MULT)

            # ---- offloaded xcorr (g0, ob0) on Vector: per-tap MAC ----
            # DMA issue is deferred: an offload dma_start waits on the Vector
            # engine, and the in-order sync queue would head-of-line-block
            # every PE-chunk dma emitted after it.
            offl_xo = {}

            def offload_xcorr(s):
                xo = xov_pool.tile([128, 961], f32, tag="xov", name=f"xov{s}")
                offl_xo[s] = xo
                xov = xo[:].rearrange("p (h w) -> p h w", h=31, w=31)
                sov = spout[(s, 0)][:].rearrange("p (h w) -> p h w", h=35, w=35)
                for t in range(25):
                    dy, dx = t // 5, t % 5
                    src = sov[:, dy:dy + 31, dx:dx + 31]
                    kcol = kf[0][:, s * 25 + t:s * 25 + t + 1]
                    if t == 0:
                        nc.vector.tensor_scalar(xov, src, kcol, None, MULT)
                    else:
                        nc.vector.scalar_tensor_tensor(
                            xov, src, kcol, xov, MULT, ADD)

            def offload_dma(s):
                xo = offl_xo[s]
                for q in range(2):
                    nc.sync.dma_start(outp_d[0, 0, s, 64 * q:64 * q + 64, :],
                                      xo[64 * q:64 * q + 64, :])

            # ---- PE xcorr for one sample pair (64x64 tiling) ----
            def xcorr_pair(g, ob, pair, last=False):
                sA, sB = g * 4 + 2 * pair, g * 4 + 2 * pair + 1
                rA, rB = 2 * pair, 2 * pair + 1
                xoA = xout_pool.tile([128, 961], f32, tag="xo",
                                     name=f"xo{g}{ob}{rA}")
                xoB = xout_pool.tile([128, 961], f32, tag="xo",
                                     name=f"xo{g}{ob}{rB}")
                stA, stB = strips[(sA, ob)], strips[(sB, ob)]
                sovA = spout[(sA, ob)][:].rearrange("p (h w) -> p h w", h=35, w=35)
                sovB = spout[(sB, ob)][:].rearrange("p (h w) -> p h w", h=35, w=35)
                for ci, (y0, nr, pool, ptag) in enumerate(
                        [(0, 16, psx, "xc"), (16, 15, psc, "conv")]):
                    N = nr * 31
                    pxA = pool.tile([128, 512], f32, tag=ptag, name=f"pxA{ci}")
                    pxB = pool.tile([128, 512], f32, tag=ptag, name=f"pxB{ci}")
                    for t in range(25):
                        dy, dx = t // 5, t % 5
                        ts, te = t * 64, (t + 1) * 64
                        r0, r1 = y0 + dy, y0 + dy + nr
                        nc.tensor.matmul(
                            pxA[0:64, 0:N], stA[0:64, ts:te],
                            sovA[0:64, r0:r1, dx:dx + 31],
                            start=(t == 0), stop=(t == 24),
                            tile_position=(0, 0))
                        nc.tensor.matmul(
                            pxA[64:128, 0:N], stB[64:128, ts:te],
                            sovB[64:128, r0:r1, dx:dx + 31],
                            start=(t == 0), stop=(t == 24),
                            tile_position=(64, 64))
                        nc.tensor.matmul(
                            pxB[0:64, 0:N], stA[64:128, ts:te],
                            sovA[64:128, r0:r1, dx:dx + 31],
                            start=(t == 0), stop=(t == 24),
                            tile_position=(64, 0))
                        nc.tensor.matmul(
                            pxB[64:128, 0:N], stB[0:64, ts:te],
                            sovB[0:64, r0:r1, dx:dx + 31],
                            start=(t == 0), stop=(t == 24),
                            tile_position=(0, 64))
                    nc.scalar.copy(xoA[:, y0 * 31:y0 * 31 + N], pxA[:, 0:N])
                    nc.scalar.copy(xoB[:, y0 * 31:y0 * 31 + N], pxB[:, 0:N])
                for r, xo in ((rA, xoA), (rB, xoB)):
                    if last:
                        for q in range(2):
                            eng = nc.sync if (r + q) % 2 == 0 else nc.scalar
                            eng.dma_start(
                                outp_d[g, ob, r, 64 * q:64 * q + 64, :],
                                xo[64 * q:64 * q + 64, :])
                    else:
                        eng = nc.sync if r % 2 == 0 else nc.scalar
                        eng.dma_start(outp_d[g, ob, r], xo[:])

            # ---- main schedule ----
            conv_s_sample(0)
            conv_s_sample(1)
            emit_conv_k()
            emit_strips([(0, 1), (1, 1), (2, 1), (3, 1)])
            conv_s_sample(2)
            emit_strips([(4, 0), (5, 0), (6, 0), (7, 0)])
            conv_s_sample(3)
            emit_strips([(4, 1), (5, 1), (6, 1), (7, 1)])
            offload_xcorr(0)
            offload_xcorr(1)
            offload_xcorr(2)
            offload_xcorr(3)
            conv_s_sample(4)
            conv_s_sample(5)
            conv_s_sample(6)
            conv_s_sample(7)
            xcorr_pair(0, 1, 0)
            xcorr_pair(0, 1, 1)
            offload_dma(0)
            offload_dma(1)
            xcorr_pair(1, 0, 0)
            xcorr_pair(1, 0, 1)
            offload_dma(2)
            offload_dma(3)
            xcorr_pair(1, 1, 0)
            xcorr_pair(1, 1, 1, last=True)

    nc.compile()
    return nc


def _host_prep(kernel, search, w_k, g_k, b_k, m_k, v_k, w_s, g_s, b_s, m_s, v_s):
    import ml_dtypes
    bf = ml_dtypes.bfloat16

    def fold(w, g, b, m, v):
        scale = g / np.sqrt(v + EPS)
        return (w * scale[:, None, None, None]).astype(np.float32), \
               (b - m * scale).astype(np.float32)

    wkf, bias_k = fold(w_k, g_k, b_k, m_k, v_k)
    wsf, bias_s = fold(w_s, g_s, b_s, m_s, v_s)

    def packT(w):  # [o, ci, 3, 3] -> [cb][ci(128), (ob,t,o)] bf16
        arr = w.reshape(2, 128, 2, 128, 9).transpose(2, 3, 0, 4, 1)
        return np.ascontiguousarray(arr).astype(bf).reshape(2, 128, 2304)

    wTk = packT(wkf)
    wTk = np.ascontiguousarray(np.concatenate([wTk[0], wTk[1]], axis=1))
    wTs = packT(wsf)  # DRAM tensors stay [128, 2304]; kernel slices halves
    wTs = [np.ascontiguousarray(wTs[0]), np.ascontiguousarray(wTs[1])]

    MSK = np.zeros((128, 25, 64), dtype=np.float32)
    for p in range(128):
        MSK[p, :, p % 64] = 1.0
    MSK = np.ascontiguousarray(MSK.reshape(128, 1600)).astype(bf)

    cst = np.ascontiguousarray(
        np.stack([bias_k[0:128], bias_k[128:256],
                  bias_s[0:128], bias_s[128:256]], axis=1).astype(np.float32))

    in_maps = []
    for core in range(N_CORES):
        kin = kernel[core * SPC:(core + 1) * SPC]
        sin = search[core * SPC:(core + 1) * SPC]

        Xk = np.zeros((2, 128, 9, 256), dtype=np.float32)
        for t in range(9):
            dy, dx = t // 3, t % 3
            p = kin[:, :, dy:dy + 5, dx:dx + 5].reshape(SPC, 2, 128, 25)
            Xk[:, :, t, :200] = p.transpose(1, 2, 0, 3).reshape(2, 128, 200)
        Xk = Xk.astype(bf).reshape(2, 128, 2304)
        Xk = np.ascontiguousarray(np.concatenate([Xk[0], Xk[1]], axis=1))

        Xs = np.zeros((SPC, 2, 128, 33, 34), dtype=np.float32)
        Xs[:, :, :, 1:32, 1:32] = sin.reshape(SPC, 2, 128, 31, 31)
        Xs = Xs.astype(bf).transpose(0, 2, 1, 3, 4)
        Xs = np.ascontiguousarray(Xs.reshape(SPC, 128, 2244))

        in_maps.append({
            "wTs0": wTs[0], "wTs1": wTs[1], "wTk": wTk, "xk": Xk, "xs": Xs,
            "cst": cst, "msk": MSK,
        })
    return in_maps


def kernel(kernel, search, w_k, g_k, b_k, m_k, v_k, w_s, g_s, b_s, m_s, v_s,
           _trace=False):
    global _cached_nc, last_results
    args = [np.ascontiguousarray(np.asarray(x, dtype=np.float32)) for x in
            (kernel, search, w_k, g_k, b_k, m_k, v_k, w_s, g_s, b_s, m_s, v_s)]
    if _cached_nc is None:
        _cached_nc = _build_program()
    nc = _cached_nc
    in_maps = _host_prep(*args)
    res = run_bass_kernel_spmd(nc, in_maps, core_ids=list(range(N_CORES)),
                               trace=_trace)
    last_results = res
    outs = []
    for i in range(N_CORES):
        op = res.results[i]["outp"]  # [g, ob, r, p, n]
        out = np.empty((SPC, CIN, 961), dtype=op.dtype)
        # offloaded (g0, ob0): r = sample, partitions = channels 0:128
        for s in range(4):
            out[s, 0:128] = op[0, 0, s]
        # PE pair-chunks: quadrant layout
        for g in range(2):
            for ob in range(2):
                if g == 0 and ob == 0:
                    continue
                for r in range(4):
                    pair, side = r // 2, r % 2
                    for q in range(2):
                        smp = g * 4 + 2 * pair + q
                        ch = ob * 128 + (q ^ side) * 64
                        out[smp, ch:ch + 64] = op[g, ob, r, 64 * q:64 * q + 64]
        outs.append(out.reshape(SPC, CIN, 31, 31))
    out = np.concatenate(outs, axis=0)
    return np.ascontiguousarray(out.astype(np.float32))


# revision 30
# speedup vs baseline: 1.0149x; 1.0149x over previous
"""Trainium2 Bass kernel for DepthwiseXCorrAug.

Computes, for B=64 samples sharded 8-per-core across 8 NeuronCores:
  k = relu(bn(conv3x3_valid(kernel_in, w_k)))     # [B,256,5,5]
  s = relu(bn(conv3x3_same(search_in, w_s)))      # [B,256,31,31]
  out = per-sample per-channel xcorr(s, k), pad 2 # [B,256,31,31]

Device strategy (per core):
  - conv branches as bf16 matmuls over (ci-block x 3x3-tap) accumulated in
    fp32 PSUM; BN folded into weights on host, bias+ReLU by ScalarE on
    eviction into zero-bordered bf16 spout tiles.
  - depthwise xcorr: the (g0, ob0) quarter runs on the Vector engine as
    per-tap multiply-accumulate (scalar_tensor_tensor) into SBUF; the rest
    runs on the PE as bf16 diagonal-weight matmuls in 64x64 tiling at
    sample-pair granularity.
  - diagonal strips built on GpSimd as ONE broadcast tensor_tensor per
    (sample, ob): kf broadcast (0-stride) x tiled diag mask.
  - DMA: partition-major packed layouts (multi-KB descriptors); outputs
    drain into a packed [g,ob,r,128,961] layout across both HWDGE rings.
"""

import sys

sys.path.insert(0, "/opt/trn_rl_repo")

import numpy as np

import concourse.bass as bass
import concourse.mybir as mybir
import concourse.tile as tile
from concourse import bacc
from concourse.bass_utils import run_bass_kernel_spmd

EPS = 1e-5
N_CORES = 8
B, CIN, HID = 64, 256, 256
SPC = B // N_CORES  # samples per core

_cached_nc = None
last_results = None  # set by kernel(); used by test harness for profiling


def _build_program():
    f32 = mybir.dt.float32
    bf16 = mybir.dt.bfloat16
    RELU = mybir.ActivationFunctionType.Relu
    MULT = mybir.AluOpType.mult
    ADD = mybir.AluOpType.add

    nc = bacc.Bacc("TRN2", target_bir_lowering=False, debug=False,
                   num_devices=N_CORES)

    wTs_d = [nc.dram_tensor(f"wTs{cb}", [128, 2304], bf16,
                            kind="ExternalInput").ap() for cb in range(2)]
    wTk_d = nc.dram_tensor("wTk", [128, 4608], bf16, kind="ExternalInput").ap()
    xk_d = nc.dram_tensor("xk", [128, 4608], bf16, kind="ExternalInput").ap()
    xs_d = nc.dram_tensor("xs", [SPC, 128, 2244], bf16, kind="ExternalInput").ap()
    cst_d = nc.dram_tensor("cst", [128, 4], f32, kind="ExternalInput").ap()
    msk_d = nc.dram_tensor("msk", [128, 1600], bf16, kind="ExternalInput").ap()
    outp_d = nc.dram_tensor("outp", [2, 2, 4, 128, 961], f32,
                            kind="ExternalOutput").ap()

    with tile.TileContext(nc) as tc:
        with tc.tile_pool(name="wp", bufs=1) as wp, \
             tc.tile_pool(name="spin", bufs=8) as spin_pool, \
             tc.tile_pool(name="xop", bufs=10) as xout_pool, \
             tc.tile_pool(name="xov", bufs=4) as xov_pool, \
             tc.tile_pool(name="psc", bufs=4, space="PSUM") as psc, \
             tc.tile_pool(name="psx", bufs=4, space="PSUM") as psx:

            # ---- persistent tiles ----
            wTs = [[wp.tile([128, 1152], bf16, tag=f"wTs{cb}{ob}",
                            name=f"wTs{cb}{ob}") for ob in range(2)]
                   for cb in range(2)]
            wTk = wp.tile([128, 4608], bf16, tag="wTk", name="wTk")
            xk = wp.tile([128, 4608], bf16, tag="xk", name="xk")
            cst = wp.tile([128, 4], f32, tag="cst", name="cst")
            msk = wp.tile([128, 1600], bf16, tag="msk", name="msk")
            kf = [wp.tile([128, 200], f32, tag=f"kf{ob}", name=f"kf{ob}")
                  for ob in range(2)]
            spout = {}
            strips = {}
            for s in range(SPC):
                for ob in range(2):
                    spout[(s, ob)] = wp.tile(
                        [128, 35 * 35], bf16, tag=f"sp{s}_{ob}", name=f"sp{s}_{ob}")
                    if not (ob == 0 and s < 4):  # (g0, ob0) runs on Vector
                        strips[(s, ob)] = wp.tile(
                            [128, 1600], bf16, tag=f"st{s}_{ob}",
                            name=f"st{s}_{ob}")

            # zero spout borders while the first DMAs are in flight
            for s in range(SPC):
                for ob in range(2):
                    nc.gpsimd.memset(spout[(s, ob)][:], 0.0)

            # ---- input DMA: head-critical first on the sync ring ----
            spin = {}

            def alloc_spin(s):
                spin[s] = spin_pool.tile([128, 2244], bf16, tag="spin",
                                         name=f"spin{s}")

            spin0 = [wp.tile([128, 1122], bf16, tag=f"spin0{cb}",
                             name=f"spin0{cb}") for cb in range(2)]
            nc.sync.dma_start(wTs[0][0][:], wTs_d[0][:, 0:1152])
            nc.sync.dma_start(spin0[0][:], xs_d[0][:, 0:1122])
            nc.sync.dma_start(wTs[1][0][:], wTs_d[1][:, 0:1152])
            nc.sync.dma_start(spin0[1][:], xs_d[0][:, 1122:2244])
            nc.sync.dma_start(wTs[0][1][:], wTs_d[0][:, 1152:2304])
            nc.sync.dma_start(cst[:], cst_d)
            nc.sync.dma_start(wTs[1][1][:], wTs_d[1][:, 1152:2304])
            alloc_spin(1)
            nc.sync.dma_start(spin[1][:], xs_d[1])
            nc.scalar.dma_start(wTk[:], wTk_d)
            nc.scalar.dma_start(xk[:], xk_d)
            nc.scalar.dma_start(msk[:], msk_d)
            for s in range(2, SPC):
                alloc_spin(s)
                nc.sync.dma_start(spin[s][:], xs_d[s])

            def spin_view(s, cb):
                if s == 0:
                    return spin0[cb][:].rearrange(
                        "p (h w) -> p h w", h=33, w=34)
                return spin[s][:, cb * 1122:(cb + 1) * 1122].rearrange(
                    "p (h w) -> p h w", h=33, w=34)

            # ---- conv_s for one sample ----
            def conv_s_sample(s):
                for ob in range(2):
                    p0 = psc.tile([128, 512], f32, tag="conv", name=f"c{s}{ob}0")
                    p1 = psc.tile([128, 512], f32, tag="conv", name=f"c{s}{ob}1")
                    idx = 0
                    for cb in range(2):
                        v = spin_view(s, cb)
                        for t in range(9):
                            dy, dx = t // 3, t % 3
                            lhsT = wTs[cb][ob][:, t * 128:(t + 1) * 128]
                            nc.tensor.matmul(
                                p0[:, 0:496], lhsT,
                                v[:, dy:dy + 16, dx:dx + 31],
                                start=(idx == 0), stop=(idx == 17))
                            nc.tensor.matmul(
                                p1[:, 0:465], lhsT,
                                v[:, 16 + dy:16 + dy + 15, dx:dx + 31],
                                start=(idx == 0), stop=(idx == 17))
                            idx += 1
                    sov = spout[(s, ob)][:].rearrange(
                        "p (h w) -> p h w", h=35, w=35)
                    nc.scalar.activation(
                        sov[:, 2:18, 2:33],
                        p0[:, 0:496].rearrange("p (h w) -> p h w", h=16, w=31),
                        RELU, bias=cst[:, 2 + ob:3 + ob], scale=1.0)
                    nc.scalar.activation(
                        sov[:, 18:33, 2:33],
                        p1[:, 0:465].rearrange("p (h w) -> p h w", h=15, w=31),
                        RELU, bias=cst[:, 2 + ob:3 + ob], scale=1.0)

            # ---- conv_k: all 8 samples batched on the free dim ----
            def emit_conv_k():
                for ob in range(2):
                    pk = psc.tile([128, 512], f32, tag="conv", name=f"pk{ob}")
                    idx = 0
                    for cb in range(2):
                        for t in range(9):
                            nc.tensor.matmul(
                                pk[:, 0:200],
                                wTk[:, cb * 2304 + ob * 1152 + t * 128:
                                    cb * 2304 + ob * 1152 + (t + 1) * 128],
                                xk[:, cb * 2304 + t * 256:
                                   cb * 2304 + t * 256 + 200],
                                start=(idx == 0), stop=(idx == 17))
                            idx += 1
                    nc.scalar.activation(kf[ob][:], pk[:, 0:200], RELU,
                                         bias=cst[:, ob:ob + 1], scale=1.0)

            # ---- strips on GpSimd: one broadcast mult per (s, ob) ----
            mskv = msk[:].rearrange("p (t c) -> p t c", t=25, c=64)

            def emit_strips(units):
                for (s, ob) in units:
                    st = strips[(s, ob)]
                    kv = kf[ob][:, s * 25:(s + 1) * 25].unsqueeze(2) \
                        .broadcast_to([128, 25, 64])
                    nc.gpsimd.tensor_tensor(
                        st[:].rearrange("p (t c) -> p t c", t=25, c=64),
                        kv, mskv, MULT)

            # ---- offloaded xcorr (g0, ob0) on Vector: per-tap MAC ----
            def offload_xcorr(s):
                xo = xov_pool.tile([128, 961], f32, tag="xov", name=f"xov{s}")
                xov = xo[:].rearrange("p (h w) -> p h w", h=31, w=31)
                sov = spout[(s, 0)][:].rearrange("p (h w) -> p h w", h=35, w=35)
                for t in range(25):
                    dy, dx = t // 5, t % 5
                    src = sov[:, dy:dy + 31, dx:dx + 31]
                    kcol = kf[0][:, s * 25 + t:s * 25 + t + 1]
                    if t == 0:
                        nc.vector.tensor_scalar(xov, src, kcol, None, MULT)
                    else:
                        nc.vector.scalar_tensor_tensor(
                            xov, src, kcol, xov, MULT, ADD)
                for q in range(2):
                    nc.sync.dma_start(outp_d[0, 0, s, 64 * q:64 * q + 64, :],
                                      xo[64 * q:64 * q + 64, :])

            # ---- PE xcorr for one sample pair (64x64 tiling) ----
            def xcorr_pair(g, ob, pair, last=False):
                sA, sB = g * 4 + 2 * pair, g * 4 + 2 * pair + 1
                rA, rB = 2 * pair, 2 * pair + 1
                xoA = xout_pool.tile([128, 961], f32, tag="xo",
                                     name=f"xo{g}{ob}{rA}")
                xoB = xout_pool.tile([128, 961], f32, tag="xo",
                                     name=f"xo{g}{ob}{rB}")
                stA, stB = strips[(sA, ob)], strips[(sB, ob)]
                sovA = spout[(sA, ob)][:].rearrange("p (h w) -> p h w", h=35, w=35)
                sovB = spout[(sB, ob)][:].rearrange("p (h w) -> p h w", h=35, w=35)
                for ci, (y0, nr, pool, ptag) in enumerate(
                        [(0, 16, psx, "xc"), (16, 15, psc, "conv")]):
                    N = nr * 31
                    pxA = pool.tile([128, 512], f32, tag=ptag, name=f"pxA{ci}")
                    pxB = pool.tile([128, 512], f32, tag=ptag, name=f"pxB{ci}")
                    for t in range(25):
                        dy, dx = t // 5, t % 5
                        ts, te = t * 64, (t + 1) * 64
                        r0, r1 = y0 + dy, y0 + dy + nr
                        nc.tensor.matmul(
                            pxA[0:64, 0:N], stA[0:64, ts:te],
                            sovA[0:64, r0:r1, dx:dx + 31],
                            start=(t == 0), stop=(t == 24),
                            tile_position=(0, 0))
                        nc.tensor.matmul(
                            pxA[64:128, 0:N], stB[64:128, ts:te],
                            sovB[64:128, r0:r1, dx:dx + 31],
                            start=(t == 0), stop=(t == 24),
                            tile_position=(64, 64))
                        nc.tensor.matmul(
                            pxB[0:64, 0:N], stA[64:128, ts:te],
                            sovA[64:128, r0:r1, dx:dx + 31],
                            start=(t == 0), stop=(t == 24),
                            tile_position=(64, 0))
                        nc.tensor.matmul(
                            pxB[64:128, 0:N], stB[0:64, ts:te],
                            sovB[0:64, r0:r1, dx:dx + 31],
                            start=(t == 0), stop=(t == 24),
                            tile_position=(0, 64))
                    nc.scalar.copy(xoA[:, y0 * 31:y0 * 31 + N], pxA[:, 0:N])
                    nc.scalar.copy(xoB[:, y0 * 31:y0 * 31 + N], pxB[:, 0:N])
                for r, xo in ((rA, xoA), (rB, xoB)):
                    if last:
                        for q in range(2):
                            eng = nc.sync if (r + q) % 2 == 0 else nc.scalar
                            eng.dma_start(
                                outp_d[g, ob, r, 64 * q:64 * q + 64, :],
                                xo[64 * q:64 * q + 64, :])
                    else:
                        nc.sync.dma_start(outp_d[g, ob, r], xo[:])

            # ---- main schedule ----
            conv_s_sample(0)
            conv_s_sample(1)
            emit_conv_k()
            emit_strips([(0, 1), (1, 1), (2, 1), (3, 1)])
            conv_s_sample(2)
            emit_strips([(4, 0), (5, 0), (6, 0), (7, 0)])
            conv_s_sample(3)
            emit_strips([(4, 1), (5, 1), (6, 1), (7, 1)])
            offload_xcorr(0)
            offload_xcorr(1)
            offload_xcorr(2)
            offload_xcorr(3)
            conv_s_sample(4)
            conv_s_sample(5)
            conv_s_sample(6)
            conv_s_sample(7)
            xcorr_pair(0, 1, 0)
            xcorr_pair(0, 1, 1)
            xcorr_pair(1, 0, 0)
            xcorr_pair(1, 0, 1)
            xcorr_pair(1, 1, 0)
            xcorr_pair(1, 1, 1, last=True)

    nc.compile()
    return nc


def _host_prep(kernel, search, w_k, g_k, b_k, m_k, v_k, w_s, g_s, b_s, m_s, v_s):
    import ml_dtypes
    bf = ml_dtypes.bfloat16

    def fold(w, g, b, m, v):
        scale = g / np.sqrt(v + EPS)
        return (w * scale[:, None, None, None]).astype(np.float32), \
               (b - m * scale).astype(np.float32)

    wkf, bias_k = fold(w_k, g_k, b_k, m_k, v_k)
    wsf, bias_s = fold(w_s, g_s, b_s, m_s, v_s)

    def packT(w):  # [o, ci, 3, 3] -> [cb][ci(128), (ob,t,o)] bf16
        arr = w.reshape(2, 128, 2, 128, 9).transpose(2, 3, 0, 4, 1)
        return np.ascontiguousarray(arr).astype(bf).reshape(2, 128, 2304)

    wTk = packT(wkf)
    wTk = np.ascontiguousarray(np.concatenate([wTk[0], wTk[1]], axis=1))
    wTs = packT(wsf)  # DRAM tensors stay [128, 2304]; kernel slices halves
    wTs = [np.ascontiguousarray(wTs[0]), np.ascontiguousarray(wTs[1])]

    MSK = np.zeros((128, 25, 64), dtype=np.float32)
    for p in range(128):
        MSK[p, :, p % 64] = 1.0
    MSK = np.ascontiguousarray(MSK.reshape(128, 1600)).astype(bf)

    cst = np.ascontiguousarray(
        np.stack([bias_k[0:128], bias_k[128:256],
                  bias_s[0:128], bias_s[128:256]], axis=1).astype(np.float32))

    in_maps = []
    for core in range(N_CORES):
        kin = kernel[core * SPC:(core + 1) * SPC]
        sin = search[core * SPC:(core + 1) * SPC]

        Xk = np.zeros((2, 128, 9, 256), dtype=np.float32)
        for t in range(9):
            dy, dx = t // 3, t % 3
            p = kin[:, :, dy:dy + 5, dx:dx + 5].reshape(SPC, 2, 128, 25)
            Xk[:, :, t, :200] = p.transpose(1, 2, 0, 3).reshape(2, 128, 200)
        Xk = Xk.astype(bf).reshape(2, 128, 2304)
        Xk = np.ascontiguousarray(np.concatenate([Xk[0], Xk[1]], axis=1))

        Xs = np.zeros((SPC, 2, 128, 33, 34), dtype=np.float32)
        Xs[:, :, :, 1:32, 1:32] = sin.reshape(SPC, 2, 128, 31, 31)
        Xs = Xs.astype(bf).transpose(0, 2, 1, 3, 4)
        Xs = np.ascontiguousarray(Xs.reshape(SPC, 128, 2244))

        in_maps.append({
            "wTs0": wTs[0], "wTs1": wTs[1], "wTk": wTk, "xk": Xk, "xs": Xs,
            "cst": cst, "msk": MSK,
        })
    return in_maps


def kernel(kernel, search, w_k, g_k, b_k, m_k, v_k, w_s, g_s, b_s, m_s, v_s,
           _trace=False):
    global _cached_nc, last_results
    args = [np.ascontiguousarray(np.asarray(x, dtype=np.float32)) for x in
            (kernel, search, w_k, g_k, b_k, m_k, v_k, w_s, g_s, b_s, m_s, v_s)]
    if _cached_nc is None:
        _cached_nc = _build_program()
    nc = _cached_nc
    in_maps = _host_prep(*args)
    res = run_bass_kernel_spmd(nc, in_maps, core_ids=list(range(N_CORES)),
                               trace=_trace)
    last_results = res
    outs = []
    for i in range(N_CORES):
        op = res.results[i]["outp"]  # [g, ob, r, p, n]
        out = np.empty((SPC, CIN, 961), dtype=op.dtype)
        # offloaded (g0, ob0): r = sample, partitions = channels 0:128
        for s in range(4):
            out[s, 0:128] = op[0, 0, s]
        # PE pair-chunks: quadrant layout
        for g in range(2):
            for ob in range(2):
                if g == 0 and ob == 0:
                    continue
                for r in range(4):
                    pair, side = r // 2, r % 2
                    for q in range(2):
                        smp = g * 4 + 2 * pair + q
                        ch = ob * 128 + (q ^ side) * 64
                        out[smp, ch:ch + 64] = op[g, ob, r, 64 * q:64 * q + 64]
        outs.append(out.reshape(SPC, CIN, 31, 31))
    out = np.concatenate(outs, axis=0)
    return np.ascontiguousarray(out.astype(np.float32))
